# revision 3
# baseline (speedup 1.0000x reference)
"""MoE layer (top-2 of 8 experts, SwiGLU FFN) on 8 Trainium2 NeuronCores.

Strategy (per spec sharding_hint, expert-parallel):
  Launch 1 (data-parallel router): the 4096 tokens are sharded 512/core;
    each core computes its router logits tile in fp32 on the PE.
  Host dispatch: softmax/top-2/gates + per-expert token index lists are
    derived from the device logits (pure routing decisions + the tiny
    scalar loss epilogue).
  Launch 2 (expert-parallel FFN): core e holds expert e's w1/w3/w2 (bf16)
    and its gathered tokens (bf16, transposed layout [D, C]); computes
    yT = (silu(x@w1) * (x@w3)) @ w2 * gate fully on-device.
  Host combine: scatter-add the two expert contributions per token.

All heavy math runs on-device; the host only routes/gathers/combines.
"""

import numpy as np
import ml_dtypes

import concourse.bass as bass
import concourse.tile as tile
from concourse import bacc, mybir
from concourse.bass_utils import run_bass_kernel_spmd

# Problem shapes (hardcoded per contract)
B, S, D, F, E = 2, 2048, 768, 2048, 8
N_TOK = B * S            # 4096
TOP_K = 2
AUX_COEF = 0.01
Z_COEF = 0.001
N_CORES = 8
P = 128                  # SBUF partitions
KD = D // P              # 6  k-tiles over D
KF = F // P              # 16 k-tiles over F
SHARD = N_TOK // N_CORES # 512 tokens/core in the router launch

BF16 = mybir.dt.bfloat16
F32 = mybir.dt.float32

_router_cache = {}
_ffn_cache = {}

# Populated on every kernel() call; test harnesses may read these to report
# HW exec time when NTFF tracing is enabled (BASS_TRACE=1).
LAST_RESULTS = {}
LAST_EXEC_NS = None


def _build_router_nc():
    """Data-parallel router: logits[tok, E] = xT.T @ router_w in fp32."""
    nc = bacc.Bacc("TRN2", target_bir_lowering=False, debug=False,
                   num_devices=N_CORES)
    xT = nc.dram_tensor("xT", [D, SHARD], F32, kind="ExternalInput").ap()
    rw = nc.dram_tensor("rw", [D, E], F32, kind="ExternalInput").ap()
    logits = nc.dram_tensor("logits", [SHARD, E], F32,
                            kind="ExternalOutput").ap()

    with tile.TileContext(nc) as tc:
        with (
            tc.tile_pool(name="sb", bufs=2) as sb,
            tc.tile_pool(name="ps", bufs=4, space="PSUM") as ps,
        ):
            rw_sb = sb.tile([P, KD, E], F32, tag="rw")
            nc.sync.dma_start(rw_sb, rw.rearrange("(t p) e -> p t e", p=P))
            xT_sb = sb.tile([P, KD, SHARD], F32, tag="x")
            nc.sync.dma_start(xT_sb, xT.rearrange("(t p) n -> p t n", p=P))

            for tt in range(SHARD // P):
                ps_l = ps.tile([P, E], F32, tag="psl")
                for k in range(KD):
                    nc.tensor.matmul(
                        ps_l,
                        lhsT=xT_sb[:, k, tt * P:(tt + 1) * P],
                        rhs=rw_sb[:, k, :],
                        start=(k == 0),
                        stop=(k == KD - 1),
                    )
                lg_sb = sb.tile([P, E], F32, tag="lg")
                nc.scalar.copy(lg_sb, ps_l)
                nc.sync.dma_start(logits[tt * P:(tt + 1) * P, :], lg_sb)
    nc.compile()
    return nc


def _chunks_of(c_pad):
    out = []
    off = 0
    while off < c_pad:
        n = min(512, c_pad - off)
        out.append((off, n))
        off += n
    return out


def _build_ffn_nc(c_pad):
    """Expert-parallel SwiGLU FFN over gathered tokens (transposed layout).

    Inputs per core: xgT [D, c_pad] bf16, w1/w3 [D, F] bf16, w2 [F, D] bf16,
    gate [1, c_pad] f32. Output: yT [D, c_pad] f32 (gate pre-applied).
    """
    nc = bacc.Bacc("TRN2", target_bir_lowering=False, debug=False,
                   num_devices=N_CORES)
    xgT = nc.dram_tensor("xgT", [D, c_pad], BF16, kind="ExternalInput").ap()
    w1 = nc.dram_tensor("w1", [D, F], BF16, kind="ExternalInput").ap()
    w3 = nc.dram_tensor("w3", [D, F], BF16, kind="ExternalInput").ap()
    w2 = nc.dram_tensor("w2", [F, D], BF16, kind="ExternalInput").ap()
    gate = nc.dram_tensor("gate", [1, c_pad], F32, kind="ExternalInput").ap()
    yT = nc.dram_tensor("yT", [D, c_pad], F32, kind="ExternalOutput").ap()

    chunks = _chunks_of(c_pad)

    with tile.TileContext(nc) as tc:
        with (
            tc.tile_pool(name="wsb", bufs=1) as wsb,
            tc.tile_pool(name="hsb", bufs=2) as hsb,
            tc.tile_pool(name="ysb", bufs=2) as ysb,
            tc.tile_pool(name="ps", bufs=2, space="PSUM") as ps,
        ):
            # Weights + activations resident in SBUF; per-k DMAs so the PE
            # can start on k0 while later tiles stream in.
            xg_sb = wsb.tile([P, KD, c_pad], BF16, tag="xg")
            w1_sb = wsb.tile([P, KD, F], BF16, tag="w1")
            w3_sb = wsb.tile([P, KD, F], BF16, tag="w3")
            xg_r = xgT.rearrange("(t p) c -> p t c", p=P)
            w1_r = w1.rearrange("(t p) f -> p t f", p=P)
            w3_r = w3.rearrange("(t p) f -> p t f", p=P)
            for k in range(KD):
                nc.sync.dma_start(xg_sb[:, k, :], xg_r[:, k, :])
                nc.sync.dma_start(w1_sb[:, k, :], w1_r[:, k, :])
                nc.sync.dma_start(w3_sb[:, k, :], w3_r[:, k, :])

            gate_sb = wsb.tile([P, c_pad], F32, tag="gate")
            gate_bcast = bass.AP(
                tensor=gate.tensor, offset=gate.offset,
                ap=[[0, P], gate.ap[1]],
            )
            nc.sync.dma_start(gate_sb, gate_bcast)

            w2_sb = wsb.tile([P, KF, D], BF16, tag="w2")
            w2_r = w2.rearrange("(t p) d -> p t d", p=P)
            for k in range(KF):
                nc.sync.dma_start(w2_sb[:, k, :], w2_r[:, k, :])

            yT_r = yT.rearrange("(t p) c -> p t c", p=P)

            for (coff, clen) in chunks:
                csl = slice(coff, coff + clen)
                # ---- up projections: hT[f, tok] = silu(w1.T x) * (w3.T x)
                h_sb = hsb.tile([P, KF, 512], BF16, tag="h")
                for ft in range(KF):
                    fs = slice(ft * P, (ft + 1) * P)
                    ps1 = ps.tile([P, 512], F32, tag="ps1")
                    for k in range(KD):
                        nc.tensor.matmul(
                            ps1[:, :clen],
                            lhsT=w1_sb[:, k, fs],
                            rhs=xg_sb[:, k, csl],
                            start=(k == 0), stop=(k == KD - 1),
                        )
                    ps3 = ps.tile([P, 512], F32, tag="ps3")
                    for k in range(KD):
                        nc.tensor.matmul(
                            ps3[:, :clen],
                            lhsT=w3_sb[:, k, fs],
                            rhs=xg_sb[:, k, csl],
                            start=(k == 0), stop=(k == KD - 1),
                        )
                    s_sb = hsb.tile([P, 512], F32, tag="s")
                    nc.scalar.activation(s_sb[:, :clen], ps1[:, :clen],
                                         mybir.ActivationFunctionType.Silu)
                    nc.vector.tensor_mul(h_sb[:, ft, :clen], s_sb[:, :clen],
                                         ps3[:, :clen])

                # ---- down projection: yT[d, tok] = w2.T h  (gate applied)
                y_sb = ysb.tile([P, KD, 512], F32, tag="y")
                for dt in range(KD):
                    dsl = slice(dt * P, (dt + 1) * P)
                    psy = ps.tile([P, 512], F32, tag="psy")
                    for ft in range(KF):
                        nc.tensor.matmul(
                            psy[:, :clen],
                            lhsT=w2_sb[:, ft, dsl],
                            rhs=h_sb[:, ft, :clen],
                            start=(ft == 0), stop=(ft == KF - 1),
                        )
                    nc.vector.tensor_mul(y_sb[:, dt, :clen], psy[:, :clen],
                                         gate_sb[:, csl])
                nc.sync.dma_start(yT_r[:, :, csl], y_sb[:, :, :clen])
    nc.compile()
    return nc


def kernel(x, router_w, w1, w2, w3):
    x = np.asarray(x, dtype=np.float32)
    router_w = np.asarray(router_w, dtype=np.float32)
    w1 = np.asarray(w1, dtype=np.float32)
    w2 = np.asarray(w2, dtype=np.float32)
    w3 = np.asarray(w3, dtype=np.float32)

    x_flat = x.reshape(-1, D)
    core_ids = list(range(N_CORES))

    # ---------------- Launch 1: router logits on-device ----------------
    if "nc" not in _router_cache:
        _router_cache["nc"] = _build_router_nc()
    nc_r = _router_cache["nc"]

    in_maps = []
    for c in range(N_CORES):
        shard = x_flat[c * SHARD:(c + 1) * SHARD]
        in_maps.append({
            "xT": np.ascontiguousarray(shard.T),
            "rw": router_w,
        })
    res_r = run_bass_kernel_spmd(nc_r, in_maps, core_ids)
    logits = np.concatenate(
        [res_r.results[c]["logits"] for c in range(N_CORES)], axis=0)

    # ---------------- Host: routing decisions + loss epilogue ----------------
    lmax = logits.max(axis=-1, keepdims=True)
    ex = np.exp(logits - lmax)
    probs = ex / ex.sum(axis=-1, keepdims=True)

    top1 = np.argmax(probs, axis=-1)
    pm = probs.copy()
    pm[np.arange(N_TOK), top1] = -1.0
    top2 = np.argmax(pm, axis=-1)
    wa = probs[np.arange(N_TOK), top1]
    wb = probs[np.arange(N_TOK), top2]
    den = wa + wb
    g1 = (wa / den).astype(np.float32)
    g2 = (wb / den).astype(np.float32)

    importance = probs.astype(np.float64).mean(axis=0)
    load = np.bincount(top1, minlength=E).astype(np.float64) / N_TOK
    aux_loss = np.float32(E * np.sum(importance * load) * AUX_COEF)
    z_loss = np.float32(np.mean(logits.astype(np.float64) ** 2) * Z_COEF)

    idx_lists, gate_lists = [], []
    for e in range(E):
        sel = np.where((top1 == e) | (top2 == e))[0]
        gates = np.where(top1[sel] == e, g1[sel], g2[sel]).astype(np.float32)
        idx_lists.append(sel)
        gate_lists.append(gates)

    c_max = max(len(s) for s in idx_lists)
    c_pad = max(256, -(-c_max // P) * P)

    # ---------------- Launch 2: expert-parallel FFN ----------------
    if c_pad not in _ffn_cache:
        _ffn_cache[c_pad] = _build_ffn_nc(c_pad)
    nc_f = _ffn_cache[c_pad]

    bf = ml_dtypes.bfloat16
    w1_b = w1.astype(bf)
    w2_b = w2.astype(bf)
    w3_b = w3.astype(bf)

    in_maps = []
    for e in range(E):
        sel = idx_lists[e]
        xg = np.zeros((D, c_pad), dtype=bf)
        xg[:, :len(sel)] = x_flat[sel].T.astype(bf)
        gt = np.zeros((1, c_pad), dtype=np.float32)
        gt[0, :len(sel)] = gate_lists[e]
        in_maps.append({
            "xgT": xg,
            "w1": w1_b[e],
            "w3": w3_b[e],
            "w2": w2_b[e],
            "gate": gt,
        })
    res_f = run_bass_kernel_spmd(nc_f, in_maps, core_ids)

    global LAST_EXEC_NS
    LAST_RESULTS["router"] = res_r
    LAST_RESULTS["ffn"] = res_f
    if res_r.exec_time_ns is not None or res_f.exec_time_ns is not None:
        LAST_EXEC_NS = (res_r.exec_time_ns or 0) + (res_f.exec_time_ns or 0)

    # ---------------- Host: combine ----------------
    out = np.zeros((N_TOK, D), dtype=np.float32)
    for e in range(E):
        sel = idx_lists[e]
        yT = res_f.results[e]["yT"]
        out[sel] += yT[:, :len(sel)].T
    return out.reshape(B, S, D), aux_loss, z_loss


# revision 6
# speedup vs baseline: 1.0443x; 1.0443x over previous
"""MoE layer (top-2 of 8 experts, SwiGLU FFN) on 8 Trainium2 NeuronCores.

Strategy (per spec sharding_hint, expert-parallel):
  Launch 1 (data-parallel router): the 4096 tokens are sharded 512/core;
    each core computes its router logits tile in fp32 on the PE.
  Host dispatch: softmax/top-2/gates + per-expert token index lists are
    derived from the device logits (pure routing decisions + the tiny
    scalar loss epilogue).
  Launch 2 (expert-parallel FFN): core e holds expert e's w1/w3/w2 (bf16)
    and its gathered tokens (bf16, transposed layout [D, C]); computes
    yT = (silu(x@w1) * (x@w3)) @ w2 * gate fully on-device.
  Host combine: scatter-add the two expert contributions per token.

All heavy math runs on-device; the host only routes/gathers/combines.
"""

import numpy as np
import ml_dtypes

import concourse.bass as bass
import concourse.tile as tile
from concourse import bacc, mybir
from concourse.bass_utils import run_bass_kernel_spmd

# Problem shapes (hardcoded per contract)
B, S, D, F, E = 2, 2048, 768, 2048, 8
N_TOK = B * S            # 4096
TOP_K = 2
AUX_COEF = 0.01
Z_COEF = 0.001
N_CORES = 8
P = 128                  # SBUF partitions
KD = D // P              # 6  k-tiles over D
KF = F // P              # 16 k-tiles over F
SHARD = N_TOK // N_CORES # 512 tokens/core in the router launch
N_CHUNKS = 3             # equal token chunks per expert in the FFN launch

BF16 = mybir.dt.bfloat16
F32 = mybir.dt.float32

_router_cache = {}
_ffn_cache = {}

# Populated on every kernel() call; test harnesses may read these to report
# HW exec time when NTFF tracing is enabled (BASS_TRACE=1).
LAST_RESULTS = {}
LAST_EXEC_NS = None


def _build_router_nc():
    """Data-parallel router: logits[tok, E] = xT.T @ router_w in fp32.

    Raw-block kernel (no Tile) to avoid the Tile exit-barrier cost; the
    dataflow is a short DMA -> PE -> ACT -> DMA chain with manual sems.
    """
    nc = bacc.Bacc("TRN2", target_bir_lowering=False, debug=False,
                   num_devices=N_CORES)
    xT = nc.dram_tensor("xT", [D, SHARD], F32, kind="ExternalInput").ap()
    rw = nc.dram_tensor("rw", [D, E], F32, kind="ExternalInput").ap()
    logits = nc.dram_tensor("logits", [SHARD, E], F32,
                            kind="ExternalOutput").ap()

    TT = SHARD // P
    xT_sb = nc.alloc_sbuf_tensor("xT_sb", [P, KD, SHARD], F32).ap()
    rw_sb = nc.alloc_sbuf_tensor("rw_sb", [P, KD, E], F32).ap()
    lg_sb = nc.alloc_sbuf_tensor("lg_sb", [P, TT, E], F32).ap()
    # one PSUM bank (512 f32) per token tile so ACT reads and PE writes
    # never touch the same bank
    ps = nc.alloc_psum_tensor("ps_l", [P, TT, 512], F32).ap()

    xT_r = xT.rearrange("(t p) n -> p t n", p=P)
    rw_r = rw.rearrange("(t p) e -> p t e", p=P)

    # DMAs on different HWDGE queues complete out of order, so each input
    # DMA gets its own semaphore (a shared counter would race).
    in_sems = [nc.alloc_semaphore(f"in_sem_{k}") for k in range(KD + 1)]

    with (
        nc.Block() as block,
        nc.semaphore("dma_sem") as dma_sem,
        nc.semaphore("mm_sem") as mm_sem,
        nc.semaphore("cp_sem") as cp_sem,
    ):
        @block.sync
        def _(sync):
            sync.dma_start(rw_sb, rw_r).then_inc(in_sems[KD], 16)
            for k in range(KD):
                sync.dma_start(xT_sb[:, k, :], xT_r[:, k, :]).then_inc(
                    in_sems[k], 16)
            sync.wait_ge(cp_sem, TT)
            sync.dma_start(
                logits.rearrange("(t p) e -> p t e", p=P), lg_sb,
            ).then_inc(dma_sem, 16)
            sync.wait_ge(dma_sem, 16)

        @block.tensor
        def _(tensor):
            tensor.wait_ge(in_sems[KD], 16)
            for tt in range(TT):
                for k in range(KD):
                    if tt == 0:
                        tensor.wait_ge(in_sems[k], 16)
                    mm = tensor.matmul(
                        ps[:, tt, :E],
                        lhsT=xT_sb[:, k, tt * P:(tt + 1) * P],
                        rhs=rw_sb[:, k, :],
                        start=(k == 0),
                        stop=(k == KD - 1),
                    )
                    if k == KD - 1:
                        mm.then_inc(mm_sem, 1)

        @block.scalar
        def _(scalar):
            for tt in range(TT):
                scalar.wait_ge(mm_sem, tt + 1)
                scalar.copy(lg_sb[:, tt, :], ps[:, tt, :E]).then_inc(
                    cp_sem, 1)

    nc.compile()
    return nc


def _build_ffn_nc(c_pad):
    """Expert-parallel SwiGLU FFN over gathered tokens (transposed layout).

    Inputs per core: xgT [D, c_pad] bf16, w1/w3 [D, F] bf16, w2 [F, D] bf16,
    gate [1, c_pad] f32. Output: yT [D, c_pad] f32 (gate pre-applied).
    c_pad must be divisible by N_CHUNKS; chunk length <= 512 (PSUM bank).
    """
    nc = bacc.Bacc("TRN2", target_bir_lowering=False, debug=False,
                   num_devices=N_CORES)
    xgT = nc.dram_tensor("xgT", [D, c_pad], BF16, kind="ExternalInput").ap()
    w1 = nc.dram_tensor("w1", [D, F], BF16, kind="ExternalInput").ap()
    w3 = nc.dram_tensor("w3", [D, F], BF16, kind="ExternalInput").ap()
    w2 = nc.dram_tensor("w2", [F, D], BF16, kind="ExternalInput").ap()
    gate = nc.dram_tensor("gate", [1, c_pad], F32, kind="ExternalInput").ap()
    yT = nc.dram_tensor("yT", [D, c_pad], F32, kind="ExternalOutput").ap()

    clen = c_pad // N_CHUNKS
    assert clen * N_CHUNKS == c_pad and clen <= 512

    with tile.TileContext(nc) as tc:
        with (
            tc.tile_pool(name="wsb", bufs=1) as wsb,
            tc.tile_pool(name="hsb", bufs=2) as hsb,
            tc.tile_pool(name="ysb", bufs=2) as ysb,
            tc.tile_pool(name="ps", bufs=2, space="PSUM") as ps,
        ):
            # Resident SBUF tensors.  DMAs are split finely and emitted in
            # PE consume-order so matmuls can start within ~2us and the
            # HAM clock-gate warms early: xg(chunk0), then w1/w3 per
            # (k, f-quarter), then w2, then remaining xg chunks.
            xg_sb = wsb.tile([P, KD, c_pad], BF16, tag="xg")
            w1_sb = wsb.tile([P, KD, F], BF16, tag="w1")
            w3_sb = wsb.tile([P, KD, F], BF16, tag="w3")
            xg_r = xgT.rearrange("(t p) c -> p t c", p=P)
            w1_r = w1.rearrange("(t p) f -> p t f", p=P)
            w3_r = w3.rearrange("(t p) f -> p t f", p=P)

            for k in range(KD):
                nc.sync.dma_start(xg_sb[:, k, :clen], xg_r[:, k, :clen])
            FQ = 512  # f-quarter width for weight streaming
            for fq in range(0, F, FQ):
                fsl = slice(fq, fq + FQ)
                for k in range(KD):
                    nc.sync.dma_start(w1_sb[:, k, fsl], w1_r[:, k, fsl])
                    nc.sync.dma_start(w3_sb[:, k, fsl], w3_r[:, k, fsl])

            gate_sb = wsb.tile([P, c_pad], F32, tag="gate")
            gate_bcast = bass.AP(
                tensor=gate.tensor, offset=gate.offset,
                ap=[[0, P], gate.ap[1]],
            )
            nc.sync.dma_start(gate_sb, gate_bcast)

            w2_sb = wsb.tile([P, KF, D], BF16, tag="w2")
            w2_r = w2.rearrange("(t p) d -> p t d", p=P)
            for k in range(KF):
                nc.sync.dma_start(w2_sb[:, k, :], w2_r[:, k, :])

            for c in range(1, N_CHUNKS):
                csl = slice(c * clen, (c + 1) * clen)
                for k in range(KD):
                    nc.sync.dma_start(xg_sb[:, k, csl], xg_r[:, k, csl])

            yT_r = yT.rearrange("(t p) c -> p t c", p=P)

            for c in range(N_CHUNKS):
                csl = slice(c * clen, (c + 1) * clen)
                # ---- up projections: hT[f, tok] = silu(w1.T x) * (w3.T x)
                h_sb = hsb.tile([P, KF, clen], BF16, tag="h")
                for ft in range(KF):
                    fs = slice(ft * P, (ft + 1) * P)
                    ps1 = ps.tile([P, clen], F32, tag="ps1")
                    for k in range(KD):
                        nc.tensor.matmul(
                            ps1,
                            lhsT=w1_sb[:, k, fs],
                            rhs=xg_sb[:, k, csl],
                            start=(k == 0), stop=(k == KD - 1),
                        )
                    ps3 = ps.tile([P, clen], F32, tag="ps3")
                    for k in range(KD):
                        nc.tensor.matmul(
                            ps3,
                            lhsT=w3_sb[:, k, fs],
                            rhs=xg_sb[:, k, csl],
                            start=(k == 0), stop=(k == KD - 1),
                        )
                    s_sb = hsb.tile([P, clen], F32, tag="s")
                    nc.scalar.activation(s_sb, ps1,
                                         mybir.ActivationFunctionType.Silu)
                    nc.vector.tensor_mul(h_sb[:, ft, :], s_sb, ps3)

                # ---- down projection: yT[d, tok] = w2.T h  (gate applied)
                y_sb = ysb.tile([P, KD, clen], F32, tag="y")
                for dt in range(KD):
                    dsl = slice(dt * P, (dt + 1) * P)
                    psy = ps.tile([P, clen], F32, tag="psy")
                    for ft in range(KF):
                        nc.tensor.matmul(
                            psy,
                            lhsT=w2_sb[:, ft, dsl],
                            rhs=h_sb[:, ft, :],
                            start=(ft == 0), stop=(ft == KF - 1),
                        )
                    nc.vector.tensor_mul(y_sb[:, dt, :], psy,
                                         gate_sb[:, csl])
                nc.sync.dma_start(yT_r[:, :, csl], y_sb)
    nc.compile()
    return nc


def kernel(x, router_w, w1, w2, w3):
    x = np.asarray(x, dtype=np.float32)
    router_w = np.asarray(router_w, dtype=np.float32)
    w1 = np.asarray(w1, dtype=np.float32)
    w2 = np.asarray(w2, dtype=np.float32)
    w3 = np.asarray(w3, dtype=np.float32)

    x_flat = x.reshape(-1, D)
    core_ids = list(range(N_CORES))

    # ---------------- Launch 1: router logits on-device ----------------
    if "nc" not in _router_cache:
        _router_cache["nc"] = _build_router_nc()
    nc_r = _router_cache["nc"]

    in_maps = []
    for c in range(N_CORES):
        shard = x_flat[c * SHARD:(c + 1) * SHARD]
        in_maps.append({
            "xT": np.ascontiguousarray(shard.T),
            "rw": router_w,
        })
    res_r = run_bass_kernel_spmd(nc_r, in_maps, core_ids)
    logits = np.concatenate(
        [res_r.results[c]["logits"] for c in range(N_CORES)], axis=0)

    # ---------------- Host: routing decisions + loss epilogue ----------------
    lmax = logits.max(axis=-1, keepdims=True)
    ex = np.exp(logits - lmax)
    probs = ex / ex.sum(axis=-1, keepdims=True)

    top1 = np.argmax(probs, axis=-1)
    pm = probs.copy()
    pm[np.arange(N_TOK), top1] = -1.0
    top2 = np.argmax(pm, axis=-1)
    wa = probs[np.arange(N_TOK), top1]
    wb = probs[np.arange(N_TOK), top2]
    den = wa + wb
    g1 = (wa / den).astype(np.float32)
    g2 = (wb / den).astype(np.float32)

    importance = probs.astype(np.float64).mean(axis=0)
    load = np.bincount(top1, minlength=E).astype(np.float64) / N_TOK
    aux_loss = np.float32(E * np.sum(importance * load) * AUX_COEF)
    z_loss = np.float32(np.mean(logits.astype(np.float64) ** 2) * Z_COEF)

    idx_lists, gate_lists = [], []
    for e in range(E):
        sel = np.where((top1 == e) | (top2 == e))[0]
        gates = np.where(top1[sel] == e, g1[sel], g2[sel]).astype(np.float32)
        idx_lists.append(sel)
        gate_lists.append(gates)

    c_max = max(len(s) for s in idx_lists)
    step = 16 * N_CHUNKS
    c_pad = max(step * 8, -(-c_max // step) * step)

    # ---------------- Launch 2: expert-parallel FFN ----------------
    if c_pad not in _ffn_cache:
        _ffn_cache[c_pad] = _build_ffn_nc(c_pad)
    nc_f = _ffn_cache[c_pad]

    bf = ml_dtypes.bfloat16
    w1_b = w1.astype(bf)
    w2_b = w2.astype(bf)
    w3_b = w3.astype(bf)

    in_maps = []
    for e in range(E):
        sel = idx_lists[e]
        xg = np.zeros((D, c_pad), dtype=bf)
        xg[:, :len(sel)] = x_flat[sel].T.astype(bf)
        gt = np.zeros((1, c_pad), dtype=np.float32)
        gt[0, :len(sel)] = gate_lists[e]
        in_maps.append({
            "xgT": xg,
            "w1": w1_b[e],
            "w3": w3_b[e],
            "w2": w2_b[e],
            "gate": gt,
        })
    res_f = run_bass_kernel_spmd(nc_f, in_maps, core_ids)

    global LAST_EXEC_NS
    LAST_RESULTS["router"] = res_r
    LAST_RESULTS["ffn"] = res_f
    if res_r.exec_time_ns is not None or res_f.exec_time_ns is not None:
        LAST_EXEC_NS = (res_r.exec_time_ns or 0) + (res_f.exec_time_ns or 0)

    # ---------------- Host: combine ----------------
    out = np.zeros((N_TOK, D), dtype=np.float32)
    for e in range(E):
        sel = idx_lists[e]
        yT = res_f.results[e]["yT"]
        out[sel] += yT[:, :len(sel)].T
    return out.reshape(B, S, D), aux_loss, z_loss


# revision 10
# speedup vs baseline: 1.0710x; 1.0257x over previous
"""MoE layer (top-2 of 8 experts, SwiGLU FFN) on 8 Trainium2 NeuronCores.

Strategy (per spec sharding_hint, expert-parallel):
  Launch 1 (data-parallel router): the 4096 tokens are sharded 512/core;
    each core computes its router logits tile in fp32 on the PE.
  Host dispatch: softmax/top-2/gates + per-expert token index lists are
    derived from the device logits (pure routing decisions + the tiny
    scalar loss epilogue).
  Launch 2 (expert-parallel FFN): core e holds expert e's w1/w3/w2 (bf16)
    and its gathered tokens (bf16, transposed layout [D, C]); computes
    yT = (silu(x@w1) * (x@w3)) @ w2 * gate fully on-device.
  Host combine: scatter-add the two expert contributions per token.

All heavy math runs on-device; the host only routes/gathers/combines.
"""

import numpy as np
import ml_dtypes

import concourse.bass as bass
import concourse.tile as tile
from concourse import bacc, mybir
from concourse.bass_utils import run_bass_kernel_spmd

# Problem shapes (hardcoded per contract)
B, S, D, F, E = 2, 2048, 768, 2048, 8
N_TOK = B * S            # 4096
TOP_K = 2
AUX_COEF = 0.01
Z_COEF = 0.001
N_CORES = 8
P = 128                  # SBUF partitions
KD = D // P              # 6  k-tiles over D
KF = F // P              # 16 k-tiles over F
SHARD = N_TOK // N_CORES # 512 tokens/core in the router launch
N_CHUNKS = 3             # equal token chunks per expert in the FFN launch

BF16 = mybir.dt.bfloat16
F32 = mybir.dt.float32

_router_cache = {}
_ffn_cache = {}

# Populated on every kernel() call; test harnesses may read these to report
# HW exec time when NTFF tracing is enabled (BASS_TRACE=1).
LAST_RESULTS = {}
LAST_EXEC_NS = None


def _build_router_nc():
    """Data-parallel router: logitsT[E, tok] = router_w.T @ xT in fp32.

    Raw-block kernel (no Tile) to avoid the Tile exit-barrier cost.
    Transposed formulation: stationary = router_w tile [128, 8], moving =
    xT [128, 512] -> only KD=6 fat matmuls instead of 48 thin ones.
    """
    nc = bacc.Bacc("TRN2", target_bir_lowering=False, debug=False,
                   num_devices=N_CORES)
    xT = nc.dram_tensor("xT", [D, SHARD], F32, kind="ExternalInput").ap()
    rw = nc.dram_tensor("rw", [D, E], F32, kind="ExternalInput").ap()
    logitsT = nc.dram_tensor("logitsT", [E, SHARD], F32,
                             kind="ExternalOutput").ap()

    xT_sb = nc.alloc_sbuf_tensor("xT_sb", [P, KD, SHARD], F32).ap()
    rw_sb = nc.alloc_sbuf_tensor("rw_sb", [P, KD, E], F32).ap()
    lg_sb = nc.alloc_sbuf_tensor("lg_sb", [E, SHARD], F32).ap()
    ps = nc.alloc_psum_tensor("ps_l", [E, SHARD], F32).ap()

    xT_r = xT.rearrange("(t p) n -> p t n", p=P)
    rw_r = rw.rearrange("(t p) e -> p t e", p=P)

    # DMAs on different HWDGE queues complete out of order, so each input
    # DMA gets its own semaphore (a shared counter would race).
    in_sems = [nc.alloc_semaphore(f"in_sem_{k}") for k in range(KD + 1)]

    with (
        nc.Block(no_gpsimd_drain=True) as block,
        nc.semaphore("dma_sem") as dma_sem,
        nc.semaphore("mm_sem") as mm_sem,
        nc.semaphore("cp_sem") as cp_sem,
    ):
        @block.sync
        def _(sync):
            sync.dma_start(rw_sb, rw_r).then_inc(in_sems[KD], 16)
            for k in range(0, KD, 2):
                sync.dma_start(xT_sb[:, k, :], xT_r[:, k, :]).then_inc(
                    in_sems[k], 16)
            sync.wait_ge(cp_sem, 1)
            sync.dma_start(logitsT, lg_sb).then_inc(dma_sem, 16)
            sync.wait_ge(dma_sem, 16)

        @block.scalar
        def _(scalar):
            # second HWDGE ring: odd k tiles in parallel with sync's evens
            for k in range(1, KD, 2):
                scalar.dma_start(xT_sb[:, k, :], xT_r[:, k, :]).then_inc(
                    in_sems[k], 16)
            scalar.wait_ge(mm_sem, 1)
            scalar.copy(lg_sb, ps).then_inc(cp_sem, 1)

        @block.tensor
        def _(tensor):
            tensor.wait_ge(in_sems[KD], 16)
            for k in range(KD):
                tensor.wait_ge(in_sems[k], 16)
                mm = tensor.matmul(
                    ps,
                    lhsT=rw_sb[:, k, :],
                    rhs=xT_sb[:, k, :],
                    start=(k == 0),
                    stop=(k == KD - 1),
                )
                if k == KD - 1:
                    mm.then_inc(mm_sem, 1)

    nc.compile()
    return nc


def _build_ffn_nc(c_pad):
    """Expert-parallel SwiGLU FFN over gathered tokens (transposed layout).

    Inputs per core: xgT [D, c_pad] bf16, w1/w3 [D, F] bf16, w2 [F, D] bf16,
    gate [1, c_pad] f32. Output: yT [D, c_pad] f32 (gate pre-applied).
    c_pad must be divisible by N_CHUNKS; chunk length <= 512 (PSUM bank).
    """
    nc = bacc.Bacc("TRN2", target_bir_lowering=False, debug=False,
                   num_devices=N_CORES)
    xgT = nc.dram_tensor("xgT", [D, c_pad], BF16, kind="ExternalInput").ap()
    w1 = nc.dram_tensor("w1", [D, F], BF16, kind="ExternalInput").ap()
    w3 = nc.dram_tensor("w3", [D, F], BF16, kind="ExternalInput").ap()
    w2 = nc.dram_tensor("w2", [F, D], BF16, kind="ExternalInput").ap()
    gate = nc.dram_tensor("gate", [1, c_pad], F32, kind="ExternalInput").ap()
    yT = nc.dram_tensor("yT", [D, c_pad], F32, kind="ExternalOutput").ap()

    clen = c_pad // N_CHUNKS
    assert clen * N_CHUNKS == c_pad and clen <= 512

    with tile.TileContext(nc) as tc:
        with (
            tc.tile_pool(name="wsb", bufs=1) as wsb,
            tc.tile_pool(name="hsb", bufs=2) as hsb,
            tc.tile_pool(name="ysb", bufs=2) as ysb,
            tc.tile_pool(name="ps", bufs=2, space="PSUM") as ps,
        ):
            # Resident SBUF tensors.  Inputs stream over BOTH HWDGE rings
            # (sync + scalar) in PE consume-order — xg(chunk0) and w1
            # f-quarters on sync, w3 f-quarters and w2 halves on scalar —
            # so matmuls start within ~5us; gate/output use the SWDGE
            # (gpsimd) path so they don't contend with the input rings.
            xg_sb = wsb.tile([P, KD, c_pad], BF16, tag="xg")
            w1_sb = wsb.tile([P, KD, F], BF16, tag="w1")
            w3_sb = wsb.tile([P, KD, F], BF16, tag="w3")
            w2_sb = wsb.tile([P, KF, D], BF16, tag="w2")
            gate_sb = wsb.tile([P, c_pad], F32, tag="gate")
            xg_r = xgT.rearrange("(t p) c -> p t c", p=P)
            w1_r = w1.rearrange("(t p) f -> p t f", p=P)
            w3_r = w3.rearrange("(t p) f -> p t f", p=P)
            w2_r = w2.rearrange("(t p) d -> p t d", p=P)

            FQ = 512  # f-quarter width for weight streaming
            nc.sync.dma_start(xg_sb[:, :, :clen], xg_r[:, :, :clen])
            for fq in range(0, F, FQ):
                fsl = slice(fq, fq + FQ)
                nc.sync.dma_start(w1_sb[:, :, fsl], w1_r[:, :, fsl])
                nc.scalar.dma_start(w3_sb[:, :, fsl], w3_r[:, :, fsl])
            for c in range(1, N_CHUNKS):
                csl = slice(c * clen, (c + 1) * clen)
                nc.sync.dma_start(xg_sb[:, :, csl], xg_r[:, :, csl])
            for kh in range(0, KF, KF // 2):
                khs = slice(kh, kh + KF // 2)
                nc.scalar.dma_start(w2_sb[:, khs, :], w2_r[:, khs, :])

            gate_bcast = bass.AP(
                tensor=gate.tensor, offset=gate.offset,
                ap=[[0, P], gate.ap[1]],
            )
            nc.gpsimd.dma_start(gate_sb, gate_bcast)

            yT_r = yT.rearrange("(t p) c -> p t c", p=P)

            for c in range(N_CHUNKS):
                csl = slice(c * clen, (c + 1) * clen)
                # ---- up projections: hT[f, tok] = silu(w1.T x) * (w3.T x)
                h_sb = hsb.tile([P, KF, clen], BF16, tag="h")
                for ft in range(KF):
                    fs = slice(ft * P, (ft + 1) * P)
                    ps1 = ps.tile([P, clen], F32, tag="ps1")
                    for k in range(KD):
                        nc.tensor.matmul(
                            ps1,
                            lhsT=w1_sb[:, k, fs],
                            rhs=xg_sb[:, k, csl],
                            start=(k == 0), stop=(k == KD - 1),
                        )
                    ps3 = ps.tile([P, clen], F32, tag="ps3")
                    for k in range(KD):
                        nc.tensor.matmul(
                            ps3,
                            lhsT=w3_sb[:, k, fs],
                            rhs=xg_sb[:, k, csl],
                            start=(k == 0), stop=(k == KD - 1),
                        )
                    s_sb = hsb.tile([P, clen], F32, tag="s")
                    nc.scalar.activation(s_sb, ps1,
                                         mybir.ActivationFunctionType.Silu)
                    nc.vector.tensor_mul(h_sb[:, ft, :], s_sb, ps3)

                # ---- down projection: yT[d, tok] = w2.T h  (gate applied)
                y_sb = ysb.tile([P, KD, clen], F32, tag="y")
                for dt in range(KD):
                    dsl = slice(dt * P, (dt + 1) * P)
                    psy = ps.tile([P, clen], F32, tag="psy")
                    for ft in range(KF):
                        nc.tensor.matmul(
                            psy,
                            lhsT=w2_sb[:, ft, dsl],
                            rhs=h_sb[:, ft, :],
                            start=(ft == 0), stop=(ft == KF - 1),
                        )
                    nc.vector.tensor_mul(y_sb[:, dt, :], psy,
                                         gate_sb[:, csl])
                nc.gpsimd.dma_start(yT_r[:, :, csl], y_sb)
    nc.compile()
    return nc


def kernel(x, router_w, w1, w2, w3):
    x = np.asarray(x, dtype=np.float32)
    router_w = np.asarray(router_w, dtype=np.float32)
    w1 = np.asarray(w1, dtype=np.float32)
    w2 = np.asarray(w2, dtype=np.float32)
    w3 = np.asarray(w3, dtype=np.float32)

    x_flat = x.reshape(-1, D)
    core_ids = list(range(N_CORES))

    # ---------------- Launch 1: router logits on-device ----------------
    if "nc" not in _router_cache:
        _router_cache["nc"] = _build_router_nc()
    nc_r = _router_cache["nc"]

    in_maps = []
    for c in range(N_CORES):
        shard = x_flat[c * SHARD:(c + 1) * SHARD]
        in_maps.append({
            "xT": np.ascontiguousarray(shard.T),
            "rw": router_w,
        })
    res_r = run_bass_kernel_spmd(nc_r, in_maps, core_ids)
    logits = np.concatenate(
        [res_r.results[c]["logitsT"].T for c in range(N_CORES)], axis=0)

    # ---------------- Host: routing decisions + loss epilogue ----------------
    lmax = logits.max(axis=-1, keepdims=True)
    ex = np.exp(logits - lmax)
    probs = ex / ex.sum(axis=-1, keepdims=True)

    top1 = np.argmax(probs, axis=-1)
    pm = probs.copy()
    pm[np.arange(N_TOK), top1] = -1.0
    top2 = np.argmax(pm, axis=-1)
    wa = probs[np.arange(N_TOK), top1]
    wb = probs[np.arange(N_TOK), top2]
    den = wa + wb
    g1 = (wa / den).astype(np.float32)
    g2 = (wb / den).astype(np.float32)

    importance = probs.astype(np.float64).mean(axis=0)
    load = np.bincount(top1, minlength=E).astype(np.float64) / N_TOK
    aux_loss = np.float32(E * np.sum(importance * load) * AUX_COEF)
    z_loss = np.float32(np.mean(logits.astype(np.float64) ** 2) * Z_COEF)

    idx_lists, gate_lists = [], []
    for e in range(E):
        sel = np.where((top1 == e) | (top2 == e))[0]
        gates = np.where(top1[sel] == e, g1[sel], g2[sel]).astype(np.float32)
        idx_lists.append(sel)
        gate_lists.append(gates)

    c_max = max(len(s) for s in idx_lists)
    step = 16 * N_CHUNKS
    c_pad = max(step * 8, -(-c_max // step) * step)

    # ---------------- Launch 2: expert-parallel FFN ----------------
    if c_pad not in _ffn_cache:
        _ffn_cache[c_pad] = _build_ffn_nc(c_pad)
    nc_f = _ffn_cache[c_pad]

    bf = ml_dtypes.bfloat16
    w1_b = w1.astype(bf)
    w2_b = w2.astype(bf)
    w3_b = w3.astype(bf)

    in_maps = []
    for e in range(E):
        sel = idx_lists[e]
        xg = np.zeros((D, c_pad), dtype=bf)
        xg[:, :len(sel)] = x_flat[sel].T.astype(bf)
        gt = np.zeros((1, c_pad), dtype=np.float32)
        gt[0, :len(sel)] = gate_lists[e]
        in_maps.append({
            "xgT": xg,
            "w1": w1_b[e],
            "w3": w3_b[e],
            "w2": w2_b[e],
            "gate": gt,
        })
    res_f = run_bass_kernel_spmd(nc_f, in_maps, core_ids)

    global LAST_EXEC_NS
    LAST_RESULTS["router"] = res_r
    LAST_RESULTS["ffn"] = res_f
    if res_r.exec_time_ns is not None or res_f.exec_time_ns is not None:
        LAST_EXEC_NS = (res_r.exec_time_ns or 0) + (res_f.exec_time_ns or 0)

    # ---------------- Host: combine ----------------
    out = np.zeros((N_TOK, D), dtype=np.float32)
    for e in range(E):
        sel = idx_lists[e]
        yT = res_f.results[e]["yT"]
        out[sel] += yT[:, :len(sel)].T
    return out.reshape(B, S, D), aux_loss, z_loss


# revision 13
# speedup vs baseline: 1.0880x; 1.0158x over previous
"""MoE layer (top-2 of 8 experts, SwiGLU FFN) on 8 Trainium2 NeuronCores.

Strategy (per spec sharding_hint, expert-parallel):
  Launch 1 (data-parallel router): the 4096 tokens are sharded 512/core;
    each core computes its router logits tile in fp32 on the PE.
  Host dispatch: softmax/top-2/gates + per-expert token index lists are
    derived from the device logits (pure routing decisions + the tiny
    scalar loss epilogue).
  Launch 2 (expert-parallel FFN): core e holds expert e's w1/w3/w2 (bf16)
    and its gathered tokens (bf16); computes
    yT = (silu(x@w1) * (x@w3)) @ w2 * gate fully on-device.
  Host combine: scatter-add the two expert contributions per token.

All heavy math runs on-device; the host only routes/gathers/combines.
All DRAM inputs/outputs use partition-major tiled layouts ([128, ...]
with long contiguous per-partition runs) so every DMA descriptor moves
multi-KB and the transfers run at HBM line rate.
"""

import numpy as np
import ml_dtypes

import concourse.bass as bass
import concourse.tile as tile
from concourse import bacc, mybir
from concourse.bass_utils import run_bass_kernel_spmd

# Problem shapes (hardcoded per contract)
B, S, D, F, E = 2, 2048, 768, 2048, 8
N_TOK = B * S            # 4096
TOP_K = 2
AUX_COEF = 0.01
Z_COEF = 0.001
N_CORES = 8
P = 128                  # SBUF partitions
KD = D // P              # 6  k-tiles over D
KF = F // P              # 16 k-tiles over F
SHARD = N_TOK // N_CORES # 512 tokens/core in the router launch
N_CHUNKS = 3             # equal token chunks per expert in the FFN launch
NQ = 4                   # f-quarters for w1/w3 streaming
NH = 2                   # halves for w2 streaming

BF16 = mybir.dt.bfloat16
F32 = mybir.dt.float32

_router_cache = {}
_ffn_cache = {}

# Populated on every kernel() call; test harnesses may read these to report
# HW exec time when NTFF tracing is enabled (BASS_TRACE=1).
LAST_RESULTS = {}
LAST_EXEC_NS = None


def _pm(a, p=P):
    """[R, C] -> partition-major tiled [p, (R//p) * C], row r = t*p + q."""
    r, c = a.shape
    return np.ascontiguousarray(
        a.reshape(r // p, p, c).transpose(1, 0, 2).reshape(p, -1))


def _build_router_nc():
    """Data-parallel router: logitsT[E, tok] = router_w.T @ xT in fp32.

    Raw-block kernel (no Tile) to avoid the Tile exit-barrier cost.
    Transposed formulation: stationary = router_w tile [128, 8], moving =
    xT [128, 512] -> only KD=6 fat matmuls instead of 48 thin ones.
    """
    nc = bacc.Bacc("TRN2", target_bir_lowering=False, debug=False,
                   num_devices=N_CORES)
    # partition-major: xT_pm[p, k*SHARD + n] = x_shard[n, k*128+p]
    xT = nc.dram_tensor("xT", [P, KD * SHARD], F32, kind="ExternalInput").ap()
    rw = nc.dram_tensor("rw", [P, KD * E], F32, kind="ExternalInput").ap()
    logitsT = nc.dram_tensor("logitsT", [E, SHARD], F32,
                             kind="ExternalOutput").ap()

    xT_sb = nc.alloc_sbuf_tensor("xT_sb", [P, KD, SHARD], F32).ap()
    rw_sb = nc.alloc_sbuf_tensor("rw_sb", [P, KD, E], F32).ap()
    lg_sb = nc.alloc_sbuf_tensor("lg_sb", [E, SHARD], F32).ap()
    ps = nc.alloc_psum_tensor("ps_l", [E, SHARD], F32).ap()

    xT_r = xT.rearrange("p (t n) -> p t n", t=KD)
    rw_r = rw.rearrange("p (t e) -> p t e", t=KD)

    # DMAs on different HWDGE queues complete out of order, so each input
    # DMA gets its own semaphore (a shared counter would race).
    in_sems = [nc.alloc_semaphore(f"in_sem_{k}") for k in range(KD + 1)]

    with (
        nc.Block(no_gpsimd_drain=True) as block,
        nc.semaphore("dma_sem") as dma_sem,
        nc.semaphore("mm_sem") as mm_sem,
        nc.semaphore("cp_sem") as cp_sem,
    ):
        @block.sync
        def _(sync):
            sync.dma_start(rw_sb, rw_r).then_inc(in_sems[KD], 16)
            for k in range(0, KD, 2):
                sync.dma_start(xT_sb[:, k, :], xT_r[:, k, :]).then_inc(
                    in_sems[k], 16)
            sync.wait_ge(cp_sem, 1)
            sync.dma_start(logitsT, lg_sb).then_inc(dma_sem, 16)
            sync.wait_ge(dma_sem, 16)

        @block.scalar
        def _(scalar):
            # second HWDGE ring: odd k tiles in parallel with sync's evens
            for k in range(1, KD, 2):
                scalar.dma_start(xT_sb[:, k, :], xT_r[:, k, :]).then_inc(
                    in_sems[k], 16)
            scalar.wait_ge(mm_sem, 1)
            scalar.copy(lg_sb, ps).then_inc(cp_sem, 1)

        @block.tensor
        def _(tensor):
            tensor.wait_ge(in_sems[KD], 16)
            for k in range(KD):
                tensor.wait_ge(in_sems[k], 16)
                mm = tensor.matmul(
                    ps,
                    lhsT=rw_sb[:, k, :],
                    rhs=xT_sb[:, k, :],
                    start=(k == 0),
                    stop=(k == KD - 1),
                )
                if k == KD - 1:
                    mm.then_inc(mm_sem, 1)

    nc.compile()
    return nc


def _build_ffn_nc(c_pad):
    """Expert-parallel SwiGLU FFN over gathered tokens.

    Partition-major DRAM layouts (q = f-quarter, h = f-half, c = chunk):
      xg   [N_CHUNKS, P, KD*clen]   bf16   xg[c][p][k*clen+j]  = x[tok, d]
      w1/3 [NQ, P, KD*512]          bf16   w[q][p][k*512+j]    = w[k*128+p, q*512+j]
      w2   [NH, P, 8*768]           bf16   w2[h][p][i*768+d]   = w2[(h*8+i)*128+p, d]
      gate [1, c_pad]               f32    (partition-broadcast on load)
      y    [N_CHUNKS, P, KD*clen]   f32    y[c][p][k*clen+j]   = out[tok, d]
    """
    nc = bacc.Bacc("TRN2", target_bir_lowering=False, debug=False,
                   num_devices=N_CORES)
    clen = c_pad // N_CHUNKS
    assert clen * N_CHUNKS == c_pad and clen <= 512

    xg = nc.dram_tensor("xg", [N_CHUNKS, P, KD * clen], BF16,
                        kind="ExternalInput").ap()
    w1 = nc.dram_tensor("w1", [NQ, P, KD * (F // NQ)], BF16,
                        kind="ExternalInput").ap()
    w3 = nc.dram_tensor("w3", [NQ, P, KD * (F // NQ)], BF16,
                        kind="ExternalInput").ap()
    w2 = nc.dram_tensor("w2", [NH, P, (KF // NH) * D], BF16,
                        kind="ExternalInput").ap()
    gate = nc.dram_tensor("gate", [1, c_pad], F32, kind="ExternalInput").ap()
    y = nc.dram_tensor("y", [N_CHUNKS, P, KD * clen], F32,
                       kind="ExternalOutput").ap()

    FQ = F // NQ          # 512
    KH = KF // NH         # 8

    with tile.TileContext(nc) as tc:
        with (
            tc.tile_pool(name="wsb", bufs=1) as wsb,
            tc.tile_pool(name="hsb", bufs=2) as hsb,
            tc.tile_pool(name="ysb", bufs=2) as ysb,
            tc.tile_pool(name="ps", bufs=2, space="PSUM") as ps,
        ):
            # Resident SBUF tensors.  Inputs stream over BOTH HWDGE rings
            # (sync + scalar) in PE consume-order; gate/output use the
            # SWDGE (gpsimd) path so they don't contend with the rings.
            xg_sb = wsb.tile([P, N_CHUNKS, KD, clen], BF16, tag="xg")
            w1_sb = wsb.tile([P, NQ, KD, FQ], BF16, tag="w1")
            w3_sb = wsb.tile([P, NQ, KD, FQ], BF16, tag="w3")
            w2_sb = wsb.tile([P, NH, KH, D], BF16, tag="w2")
            gate_sb = wsb.tile([P, c_pad], F32, tag="gate")

            nc.sync.dma_start(
                xg_sb[:, 0], xg[0].rearrange("p (k j) -> p k j", k=KD))
            for q in range(NQ):
                w1q = w1[q].rearrange("p (k j) -> p k j", k=KD)
                w3q = w3[q].rearrange("p (k j) -> p k j", k=KD)
                nc.sync.dma_start(w1_sb[:, q], w1q)
                nc.scalar.dma_start(w3_sb[:, q], w3q)
            for c in range(1, N_CHUNKS):
                nc.sync.dma_start(
                    xg_sb[:, c], xg[c].rearrange("p (k j) -> p k j", k=KD))
            for h in range(NH):
                nc.scalar.dma_start(
                    w2_sb[:, h], w2[h].rearrange("p (i d) -> p i d", i=KH))

            gate_bcast = bass.AP(
                tensor=gate.tensor, offset=gate.offset,
                ap=[[0, P], gate.ap[1]],
            )
            nc.gpsimd.dma_start(gate_sb, gate_bcast)

            for c in range(N_CHUNKS):
                csl = slice(c * clen, (c + 1) * clen)
                # ---- up projections: hT[f, tok] = silu(w1.T x) * (w3.T x)
                h_sb = hsb.tile([P, KF, clen], BF16, tag="h")
                for ft in range(KF):
                    # quarter-major order: ft 0..3 live in q0, etc.
                    q, jj = divmod(ft, KF // NQ)
                    fs = slice(jj * P, (jj + 1) * P)
                    ps1 = ps.tile([P, clen], F32, tag="ps1")
                    for k in range(KD):
                        nc.tensor.matmul(
                            ps1,
                            lhsT=w1_sb[:, q, k, fs],
                            rhs=xg_sb[:, c, k, :],
                            start=(k == 0), stop=(k == KD - 1),
                        )
                    ps3 = ps.tile([P, clen], F32, tag="ps3")
                    for k in range(KD):
                        nc.tensor.matmul(
                            ps3,
                            lhsT=w3_sb[:, q, k, fs],
                            rhs=xg_sb[:, c, k, :],
                            start=(k == 0), stop=(k == KD - 1),
                        )
                    s_sb = hsb.tile([P, clen], F32, tag="s")
                    nc.scalar.activation(s_sb, ps1,
                                         mybir.ActivationFunctionType.Silu)
                    nc.vector.tensor_mul(h_sb[:, ft, :], s_sb, ps3)

                # ---- down projection: y[d, tok] = w2.T h  (gate applied)
                y_sb = ysb.tile([P, KD, clen], F32, tag="y")
                for dt in range(KD):
                    dsl = slice(dt * P, (dt + 1) * P)
                    psy = ps.tile([P, clen], F32, tag="psy")
                    for ft in range(KF):
                        h2, i = divmod(ft, KH)
                        nc.tensor.matmul(
                            psy,
                            lhsT=w2_sb[:, h2, i, dsl],
                            rhs=h_sb[:, ft, :],
                            start=(ft == 0), stop=(ft == KF - 1),
                        )
                    nc.vector.tensor_mul(y_sb[:, dt, :], psy,
                                         gate_sb[:, csl])
                nc.gpsimd.dma_start(
                    y[c].rearrange("p (k j) -> p k j", k=KD), y_sb)
    nc.compile()
    return nc


def kernel(x, router_w, w1, w2, w3):
    x = np.asarray(x, dtype=np.float32)
    router_w = np.asarray(router_w, dtype=np.float32)
    w1 = np.asarray(w1, dtype=np.float32)
    w2 = np.asarray(w2, dtype=np.float32)
    w3 = np.asarray(w3, dtype=np.float32)

    x_flat = x.reshape(-1, D)
    core_ids = list(range(N_CORES))

    # ---------------- Launch 1: router logits on-device ----------------
    if "nc" not in _router_cache:
        _router_cache["nc"] = _build_router_nc()
    nc_r = _router_cache["nc"]

    rw_pm = _pm(router_w)  # [P, KD*E]
    in_maps = []
    for c in range(N_CORES):
        shard = x_flat[c * SHARD:(c + 1) * SHARD]
        in_maps.append({
            "xT": _pm(np.ascontiguousarray(shard.T)),
            "rw": rw_pm,
        })
    res_r = run_bass_kernel_spmd(nc_r, in_maps, core_ids)
    logits = np.concatenate(
        [res_r.results[c]["logitsT"].T for c in range(N_CORES)], axis=0)

    # ---------------- Host: routing decisions + loss epilogue ----------------
    lmax = logits.max(axis=-1, keepdims=True)
    ex = np.exp(logits - lmax)
    probs = ex / ex.sum(axis=-1, keepdims=True)

    top1 = np.argmax(probs, axis=-1)
    pm_ = probs.copy()
    pm_[np.arange(N_TOK), top1] = -1.0
    top2 = np.argmax(pm_, axis=-1)
    wa = probs[np.arange(N_TOK), top1]
    wb = probs[np.arange(N_TOK), top2]
    den = wa + wb
    g1 = (wa / den).astype(np.float32)
    g2 = (wb / den).astype(np.float32)

    importance = probs.astype(np.float64).mean(axis=0)
    load = np.bincount(top1, minlength=E).astype(np.float64) / N_TOK
    aux_loss = np.float32(E * np.sum(importance * load) * AUX_COEF)
    z_loss = np.float32(np.mean(logits.astype(np.float64) ** 2) * Z_COEF)

    idx_lists, gate_lists = [], []
    for e in range(E):
        sel = np.where((top1 == e) | (top2 == e))[0]
        gates = np.where(top1[sel] == e, g1[sel], g2[sel]).astype(np.float32)
        idx_lists.append(sel)
        gate_lists.append(gates)

    c_max = max(len(s) for s in idx_lists)
    step = 16 * N_CHUNKS
    c_pad = max(step * 8, -(-c_max // step) * step)
    clen = c_pad // N_CHUNKS

    # ---------------- Launch 2: expert-parallel FFN ----------------
    if c_pad not in _ffn_cache:
        _ffn_cache[c_pad] = _build_ffn_nc(c_pad)
    nc_f = _ffn_cache[c_pad]

    bf = ml_dtypes.bfloat16
    FQ = F // NQ
    KH = KF // NH

    in_maps = []
    for e in range(E):
        sel = idx_lists[e]
        # xg[c][p][k*clen+j] = x[sel[c*clen+j], k*128+p]
        xg_full = np.zeros((D, c_pad), dtype=bf)
        xg_full[:, :len(sel)] = x_flat[sel].T.astype(bf)
        xg_t = (xg_full.reshape(KD, P, N_CHUNKS, clen)
                .transpose(2, 1, 0, 3).reshape(N_CHUNKS, P, KD * clen))
        w1_t = (w1[e].astype(bf).reshape(KD, P, NQ, FQ)
                .transpose(2, 1, 0, 3).reshape(NQ, P, KD * FQ))
        w3_t = (w3[e].astype(bf).reshape(KD, P, NQ, FQ)
                .transpose(2, 1, 0, 3).reshape(NQ, P, KD * FQ))
        w2_t = (w2[e].astype(bf).reshape(NH, KH, P, D)
                .transpose(0, 2, 1, 3).reshape(NH, P, KH * D))
        gt = np.zeros((1, c_pad), dtype=np.float32)
        gt[0, :len(sel)] = gate_lists[e]
        in_maps.append({
            "xg": np.ascontiguousarray(xg_t),
            "w1": np.ascontiguousarray(w1_t),
            "w3": np.ascontiguousarray(w3_t),
            "w2": np.ascontiguousarray(w2_t),
            "gate": gt,
        })
    res_f = run_bass_kernel_spmd(nc_f, in_maps, core_ids)

    global LAST_EXEC_NS
    LAST_RESULTS["router"] = res_r
    LAST_RESULTS["ffn"] = res_f
    if res_r.exec_time_ns is not None or res_f.exec_time_ns is not None:
        LAST_EXEC_NS = (res_r.exec_time_ns or 0) + (res_f.exec_time_ns or 0)

    # ---------------- Host: combine ----------------
    out = np.zeros((N_TOK, D), dtype=np.float32)
    for e in range(E):
        sel = idx_lists[e]
        ye = res_f.results[e]["y"]  # [N_CHUNKS, P, KD*clen]
        # y[c][p][k*clen+j] -> yT [D, c_pad]
        yT = (ye.reshape(N_CHUNKS, P, KD, clen).transpose(2, 1, 0, 3)
              .reshape(D, c_pad))
        out[sel] += yT[:, :len(sel)].T
    return out.reshape(B, S, D), aux_loss, z_loss


# revision 17
# speedup vs baseline: 1.1428x; 1.0504x over previous
"""MoE layer (top-2 of 8 experts, SwiGLU FFN) on 8 Trainium2 NeuronCores.

Strategy (per spec sharding_hint, expert-parallel):
  Launch 1 (data-parallel router): the 4096 tokens are sharded 512/core;
    each core computes its router logits tile in fp32 on the PE.
  Host dispatch: softmax/top-2/gates + per-expert token index lists are
    derived from the device logits (pure routing decisions + the tiny
    scalar loss epilogue).
  Launch 2 (expert-parallel FFN): core e holds expert e's w1/w3/w2 (bf16)
    and its gathered tokens (bf16); computes
    yT = (silu(x@w1) * (x@w3)) @ w2 * gate fully on-device.
  Host combine: scatter-add the two expert contributions per token.

All heavy math runs on-device; the host only routes/gathers/combines.
All DRAM inputs/outputs use partition-major tiled layouts ([128, ...]
with long contiguous per-partition runs) so every DMA descriptor moves
multi-KB and the transfers run at HBM line rate.
"""

import numpy as np
import ml_dtypes

import concourse.bass as bass
import concourse.tile as tile
from concourse import bacc, mybir
from concourse.bass_utils import run_bass_kernel_spmd

# Problem shapes (hardcoded per contract)
B, S, D, F, E = 2, 2048, 768, 2048, 8
N_TOK = B * S            # 4096
TOP_K = 2
AUX_COEF = 0.01
Z_COEF = 0.001
N_CORES = 8
P = 128                  # SBUF partitions
KD = D // P              # 6  k-tiles over D
KF = F // P              # 16 k-tiles over F
SHARD = N_TOK // N_CORES # 512 tokens/core in the router launch
N_CHUNKS = 3             # equal token chunks per expert in the FFN launch
NQ = 4                   # f-quarters for w1/w3 streaming
NH = 2                   # halves for w2 streaming

BF16 = mybir.dt.bfloat16
F32 = mybir.dt.float32

_router_cache = {}
_ffn_cache = {}

# Populated on every kernel() call; test harnesses may read these to report
# HW exec time when NTFF tracing is enabled (BASS_TRACE=1).
LAST_RESULTS = {}
LAST_EXEC_NS = None


def _pm(a, p=P):
    """[R, C] -> partition-major tiled [p, (R//p) * C], row r = t*p + q."""
    r, c = a.shape
    return np.ascontiguousarray(
        a.reshape(r // p, p, c).transpose(1, 0, 2).reshape(p, -1))


def _build_router_nc():
    """Data-parallel router: logitsT[E, tok] = router_w.T @ xT in fp32.

    Raw-block kernel (no Tile) to avoid the Tile exit-barrier cost.
    Transposed formulation: stationary = router_w tile [128, 8], moving =
    xT [128, 512] -> only KD=6 fat matmuls instead of 48 thin ones.
    """
    nc = bacc.Bacc("TRN2", target_bir_lowering=False, debug=False,
                   num_devices=N_CORES)
    # partition-major: xT_pm[p, k*SHARD + n] = x_shard[n, k*128+p]
    xT = nc.dram_tensor("xT", [P, KD * SHARD], F32, kind="ExternalInput").ap()
    rw = nc.dram_tensor("rw", [P, KD * E], F32, kind="ExternalInput").ap()
    logitsT = nc.dram_tensor("logitsT", [E, SHARD], F32,
                             kind="ExternalOutput").ap()

    xT_sb = nc.alloc_sbuf_tensor("xT_sb", [P, KD, SHARD], F32).ap()
    rw_sb = nc.alloc_sbuf_tensor("rw_sb", [P, KD, E], F32).ap()
    lg_sb = nc.alloc_sbuf_tensor("lg_sb", [E, SHARD], F32).ap()
    ps = nc.alloc_psum_tensor("ps_l", [E, SHARD], F32).ap()

    xT_r = xT.rearrange("p (t n) -> p t n", t=KD)
    rw_r = rw.rearrange("p (t e) -> p t e", t=KD)

    # DMAs on different HWDGE queues complete out of order, so each input
    # DMA gets its own semaphore (a shared counter would race).
    in_sems = [nc.alloc_semaphore(f"in_sem_{k}") for k in range(KD + 1)]

    ps_w = nc.alloc_psum_tensor("ps_warm", [P, P], F32).ap()

    with (
        nc.Block(no_gpsimd_drain=True) as block,
        nc.semaphore("dma_sem") as dma_sem,
        nc.semaphore("mm_sem") as mm_sem,
        nc.semaphore("cp_sem") as cp_sem,
    ):
        @block.sync
        def _(sync):
            sync.dma_start(rw_sb, rw_r).then_inc(in_sems[KD], 16)
            for k in range(0, KD, 2):
                sync.dma_start(xT_sb[:, k, :], xT_r[:, k, :]).then_inc(
                    in_sems[k], 16)
            sync.wait_ge(cp_sem, 1)
            sync.dma_start(logitsT, lg_sb).then_inc(dma_sem, 16)
            sync.wait_ge(dma_sem, 16)

        @block.scalar
        def _(scalar):
            # second HWDGE ring: odd k tiles in parallel with sync's evens
            for k in range(1, KD, 2):
                scalar.dma_start(xT_sb[:, k, :], xT_r[:, k, :]).then_inc(
                    in_sems[k], 16)

        @block.vector
        def _(vector):
            vector.wait_ge(mm_sem, 1)
            vector.tensor_copy(lg_sb, ps).then_inc(cp_sem, 1)

        @block.tensor
        def _(tensor):
            # warm the HAM clock-gate while the input DMAs stream: ~3.4us
            # of dummy PE activity (values are garbage, results unread)
            for _i in range(8):
                tensor.matmul(ps_w, lhsT=xT_sb[:, 0, 0:P],
                              rhs=xT_sb[:, 1, 0:P], start=True, stop=True)
            tensor.wait_ge(in_sems[KD], 16)
            for k in range(KD):
                tensor.wait_ge(in_sems[k], 16)
                mm = tensor.matmul(
                    ps,
                    lhsT=rw_sb[:, k, :],
                    rhs=xT_sb[:, k, :],
                    start=(k == 0),
                    stop=(k == KD - 1),
                )
                if k == KD - 1:
                    mm.then_inc(mm_sem, 1)

    nc.compile()
    return nc


def _build_ffn_nc(c_pad):
    """Expert-parallel SwiGLU FFN over gathered tokens.

    Partition-major DRAM layouts (q = f-quarter, h = f-half, c = chunk):
      xg   [N_CHUNKS, P, KD*clen]   bf16   xg[c][p][k*clen+j]  = x[tok, d]
      w1/3 [NQ, P, KD*512]          bf16   w[q][p][k*512+j]    = w[k*128+p, q*512+j]
      w2   [NH, P, 8*768]           bf16   w2[h][p][i*768+d]   = w2[(h*8+i)*128+p, d]
      gate [1, c_pad]               f32    (partition-broadcast on load)
      y    [N_CHUNKS, P, KD*clen]   f32    y[c][p][k*clen+j]   = out[tok, d]
    """
    nc = bacc.Bacc("TRN2", target_bir_lowering=False, debug=False,
                   num_devices=N_CORES)
    clen = c_pad // N_CHUNKS
    assert clen * N_CHUNKS == c_pad and clen <= 512

    xg = nc.dram_tensor("xg", [N_CHUNKS, P, KD * clen], BF16,
                        kind="ExternalInput").ap()
    w1 = nc.dram_tensor("w1", [NQ, P, KD * (F // NQ)], BF16,
                        kind="ExternalInput").ap()
    w3 = nc.dram_tensor("w3", [NQ, P, KD * (F // NQ)], BF16,
                        kind="ExternalInput").ap()
    w2 = nc.dram_tensor("w2", [NH, P, (KF // NH) * D], BF16,
                        kind="ExternalInput").ap()
    gate = nc.dram_tensor("gate", [1, c_pad], F32, kind="ExternalInput").ap()
    y = nc.dram_tensor("y", [N_CHUNKS, P, KD * clen], F32,
                       kind="ExternalOutput").ap()

    FQ = F // NQ          # 512
    KH = KF // NH         # 8

    with tile.TileContext(nc) as tc:
        with (
            tc.tile_pool(name="wsb", bufs=1) as wsb,
            tc.tile_pool(name="hsb", bufs=2) as hsb,
            tc.tile_pool(name="ysb", bufs=2) as ysb,
            tc.tile_pool(name="ps", bufs=2, space="PSUM") as ps,
        ):
            # Resident SBUF tensors.  Inputs stream over BOTH HWDGE rings
            # (sync + scalar) in PE consume-order; gate/output use the
            # SWDGE (gpsimd) path so they don't contend with the rings.
            xg_sb = wsb.tile([P, N_CHUNKS, KD, clen], BF16, tag="xg")
            w1_sb = wsb.tile([P, NQ, KD, FQ], BF16, tag="w1")
            w3_sb = wsb.tile([P, NQ, KD, FQ], BF16, tag="w3")
            w2_sb = wsb.tile([P, NH, KH, D], BF16, tag="w2")
            gate_sb = wsb.tile([P, c_pad], F32, tag="gate")

            # PE pre-warm: dummy matmuls on a zeroed tile keep the HAM
            # clock-gate busy while the first input DMAs stream in.
            warm_sb = hsb.tile([P, 512], BF16, tag="warm")
            nc.gpsimd.memset(warm_sb, 0)
            for _i in range(14):
                ps_w = ps.tile([P, 512], F32, tag="pswarm")
                nc.tensor.matmul(ps_w, lhsT=warm_sb[:, 0:P],
                                 rhs=warm_sb, start=True, stop=True)

            nc.sync.dma_start(
                xg_sb[:, 0], xg[0].rearrange("p (k j) -> p k j", k=KD))
            for q in range(NQ):
                w1q = w1[q].rearrange("p (k j) -> p k j", k=KD)
                w3q = w3[q].rearrange("p (k j) -> p k j", k=KD)
                nc.sync.dma_start(w1_sb[:, q], w1q)
                nc.scalar.dma_start(w3_sb[:, q], w3q)
            for c in range(1, N_CHUNKS):
                nc.sync.dma_start(
                    xg_sb[:, c], xg[c].rearrange("p (k j) -> p k j", k=KD))
            for h in range(NH):
                nc.scalar.dma_start(
                    w2_sb[:, h], w2[h].rearrange("p (i d) -> p i d", i=KH))

            gate_bcast = bass.AP(
                tensor=gate.tensor, offset=gate.offset,
                ap=[[0, P], gate.ap[1]],
            )
            nc.gpsimd.dma_start(gate_sb, gate_bcast)

            for c in range(N_CHUNKS):
                csl = slice(c * clen, (c + 1) * clen)
                # ---- up projections: hT[f, tok] = silu(w1.T x) * (w3.T x)
                h_sb = hsb.tile([P, KF, clen], BF16, tag="h")
                for ft in range(KF):
                    # quarter-major order: ft 0..3 live in q0, etc.
                    q, jj = divmod(ft, KF // NQ)
                    fs = slice(jj * P, (jj + 1) * P)
                    ps1 = ps.tile([P, clen], F32, tag="ps1")
                    for k in range(KD):
                        nc.tensor.matmul(
                            ps1,
                            lhsT=w1_sb[:, q, k, fs],
                            rhs=xg_sb[:, c, k, :],
                            start=(k == 0), stop=(k == KD - 1),
                        )
                    ps3 = ps.tile([P, clen], F32, tag="ps3")
                    for k in range(KD):
                        nc.tensor.matmul(
                            ps3,
                            lhsT=w3_sb[:, q, k, fs],
                            rhs=xg_sb[:, c, k, :],
                            start=(k == 0), stop=(k == KD - 1),
                        )
                    s_sb = hsb.tile([P, clen], F32, tag="s")
                    nc.scalar.activation(s_sb, ps1,
                                         mybir.ActivationFunctionType.Silu)
                    nc.vector.tensor_mul(h_sb[:, ft, :], s_sb, ps3)

                # ---- down projection: y[d, tok] = w2.T h  (gate applied)
                y_sb = ysb.tile([P, KD, clen], F32, tag="y")
                for dt in range(KD):
                    dsl = slice(dt * P, (dt + 1) * P)
                    psy = ps.tile([P, clen], F32, tag="psy")
                    for ft in range(KF):
                        h2, i = divmod(ft, KH)
                        nc.tensor.matmul(
                            psy,
                            lhsT=w2_sb[:, h2, i, dsl],
                            rhs=h_sb[:, ft, :],
                            start=(ft == 0), stop=(ft == KF - 1),
                        )
                    nc.vector.tensor_mul(y_sb[:, dt, :], psy,
                                         gate_sb[:, csl])
                    if c == N_CHUNKS - 1 and dt % 2 == 1:
                        # last chunk: stream the output out per d-tile pair
                        # so only a sliver of DMA remains after the last MM
                        nc.gpsimd.dma_start(
                            y[c].rearrange("p (k j) -> p k j", k=KD)
                            [:, dt - 1:dt + 1, :],
                            y_sb[:, dt - 1:dt + 1, :])
                if c < N_CHUNKS - 1:
                    nc.gpsimd.dma_start(
                        y[c].rearrange("p (k j) -> p k j", k=KD), y_sb)
    nc.compile()
    return nc


def kernel(x, router_w, w1, w2, w3):
    x = np.asarray(x, dtype=np.float32)
    router_w = np.asarray(router_w, dtype=np.float32)
    w1 = np.asarray(w1, dtype=np.float32)
    w2 = np.asarray(w2, dtype=np.float32)
    w3 = np.asarray(w3, dtype=np.float32)

    x_flat = x.reshape(-1, D)
    core_ids = list(range(N_CORES))

    # ---------------- Launch 1: router logits on-device ----------------
    if "nc" not in _router_cache:
        _router_cache["nc"] = _build_router_nc()
    nc_r = _router_cache["nc"]

    rw_pm = _pm(router_w)  # [P, KD*E]
    in_maps = []
    for c in range(N_CORES):
        shard = x_flat[c * SHARD:(c + 1) * SHARD]
        in_maps.append({
            "xT": _pm(np.ascontiguousarray(shard.T)),
            "rw": rw_pm,
        })
    res_r = run_bass_kernel_spmd(nc_r, in_maps, core_ids)
    logits = np.concatenate(
        [res_r.results[c]["logitsT"].T for c in range(N_CORES)], axis=0)

    # ---------------- Host: routing decisions + loss epilogue ----------------
    lmax = logits.max(axis=-1, keepdims=True)
    ex = np.exp(logits - lmax)
    probs = ex / ex.sum(axis=-1, keepdims=True)

    top1 = np.argmax(probs, axis=-1)
    pm_ = probs.copy()
    pm_[np.arange(N_TOK), top1] = -1.0
    top2 = np.argmax(pm_, axis=-1)
    wa = probs[np.arange(N_TOK), top1]
    wb = probs[np.arange(N_TOK), top2]
    den = wa + wb
    g1 = (wa / den).astype(np.float32)
    g2 = (wb / den).astype(np.float32)

    importance = probs.astype(np.float64).mean(axis=0)
    load = np.bincount(top1, minlength=E).astype(np.float64) / N_TOK
    aux_loss = np.float32(E * np.sum(importance * load) * AUX_COEF)
    z_loss = np.float32(np.mean(logits.astype(np.float64) ** 2) * Z_COEF)

    idx_lists, gate_lists = [], []
    for e in range(E):
        sel = np.where((top1 == e) | (top2 == e))[0]
        gates = np.where(top1[sel] == e, g1[sel], g2[sel]).astype(np.float32)
        idx_lists.append(sel)
        gate_lists.append(gates)

    c_max = max(len(s) for s in idx_lists)
    step = 4 * N_CHUNKS
    c_pad = max(384, -(-c_max // step) * step)
    clen = c_pad // N_CHUNKS

    # ---------------- Launch 2: expert-parallel FFN ----------------
    if c_pad not in _ffn_cache:
        _ffn_cache[c_pad] = _build_ffn_nc(c_pad)
    nc_f = _ffn_cache[c_pad]

    bf = ml_dtypes.bfloat16
    FQ = F // NQ
    KH = KF // NH

    in_maps = []
    for e in range(E):
        sel = idx_lists[e]
        # xg[c][p][k*clen+j] = x[sel[c*clen+j], k*128+p]
        xg_full = np.zeros((D, c_pad), dtype=bf)
        xg_full[:, :len(sel)] = x_flat[sel].T.astype(bf)
        xg_t = (xg_full.reshape(KD, P, N_CHUNKS, clen)
                .transpose(2, 1, 0, 3).reshape(N_CHUNKS, P, KD * clen))
        w1_t = (w1[e].astype(bf).reshape(KD, P, NQ, FQ)
                .transpose(2, 1, 0, 3).reshape(NQ, P, KD * FQ))
        w3_t = (w3[e].astype(bf).reshape(KD, P, NQ, FQ)
                .transpose(2, 1, 0, 3).reshape(NQ, P, KD * FQ))
        w2_t = (w2[e].astype(bf).reshape(NH, KH, P, D)
                .transpose(0, 2, 1, 3).reshape(NH, P, KH * D))
        gt = np.zeros((1, c_pad), dtype=np.float32)
        gt[0, :len(sel)] = gate_lists[e]
        in_maps.append({
            "xg": np.ascontiguousarray(xg_t),
            "w1": np.ascontiguousarray(w1_t),
            "w3": np.ascontiguousarray(w3_t),
            "w2": np.ascontiguousarray(w2_t),
            "gate": gt,
        })
    res_f = run_bass_kernel_spmd(nc_f, in_maps, core_ids)

    global LAST_EXEC_NS
    LAST_RESULTS["router"] = res_r
    LAST_RESULTS["ffn"] = res_f
    if res_r.exec_time_ns is not None or res_f.exec_time_ns is not None:
        LAST_EXEC_NS = (res_r.exec_time_ns or 0) + (res_f.exec_time_ns or 0)

    # ---------------- Host: combine ----------------
    out = np.zeros((N_TOK, D), dtype=np.float32)
    for e in range(E):
        sel = idx_lists[e]
        ye = res_f.results[e]["y"]  # [N_CHUNKS, P, KD*clen]
        # y[c][p][k*clen+j] -> yT [D, c_pad]
        yT = (ye.reshape(N_CHUNKS, P, KD, clen).transpose(2, 1, 0, 3)
              .reshape(D, c_pad))
        out[sel] += yT[:, :len(sel)].T
    return out.reshape(B, S, D), aux_loss, z_loss
